# revision 17
# baseline (speedup 1.0000x reference)
"""Trainium2 Bass kernel: LookupTransformerBlock (block-causal sparse attention).

Reference semantics (B=4, T=784, D=768, H=12, Dh=64, d_ff=3072):
  x_aug = LN1(concat(memory[:, :T], x))              # [B, 2T, D], ln1 g=1/b=0
  h     = LN_att(x_aug)  (== x_aug up to O(eps) since x_aug is normalized)
  qkv   = h @ w_qkv.T ; block-causal attention over frames of 196
  x2    = x_aug + attn_out
  out   = (x2 + FFN(LN2(x2)))[:, T:, :]

Sharding: 8 cores = (batch b in 0..3) x (query-half hf in 0..1); each core
computes its 392 output rows with K/V over all 1568 positions (data-parallel,
no collectives).  One SPMD program; per-core differences (query slice, mask
columns) are carried in input data only.

v2 design (vs fp32 baseline):
  - all big GEMM operands in bf16 (1 cycle/row on PE vs 4 for fp32 LOW_HIGH)
  - ~15 large DMAs total (weights packed into [128, 6*C] stripe tiles host-side)
  - per-token LN scale/shift broadcast via ones-row matmuls into PSUM
    (no DRAM bounce)
  - K kept un-normalized (x - mu); the per-token inv-std rs[j] is folded into
    the Exp scale operand (per-partition scalar) for scores and applied to V
    during PSUM evacuation (tensor_scalar_mul)
  - scores for both 64-row head-halves land in one 2-bank PSUM tile; ONE
    3D-AP Exp per (head-pair, j-tile); frame-A mask boundary applied by a
    vector tensor_scalar_mul zeroing pass instead of a second Exp
  - LN stats squares on the idle GpSimd engine
"""

import os
import sys
from contextlib import ExitStack

import numpy as np
import ml_dtypes

for _p in ("/opt/trn_rl_repo", os.path.expanduser("~/.axon_site/_ro/trn_rl_repo")):
    if os.path.isdir(_p) and _p not in sys.path:
        sys.path.append(_p)

import concourse.bass as bass
import concourse.bacc as bacc
import concourse.mybir as mybir
import concourse.tile as tile
from concourse.bass_utils import run_bass_kernel_spmd
from concourse.masks import make_identity

F32 = mybir.dt.float32
BF16 = mybir.dt.bfloat16
AF = mybir.ActivationFunctionType
ALU = mybir.AluOpType

B = 4
T = 784
D = 768
L = 2 * T            # 1568
NQ = 392             # query rows per core
H = 12
DH = 64
DFF = 3072
NPATCH = 196
DC = D // 128        # 6
FT = DFF // 128      # 24
NJT = 13             # j-tiles over L (12 x 128 + 32)
JSZ = [128] * 12 + [32]
LCH = [512, 512, 512, 32]
EPS = 1e-5
NCORES = 8
JLO = 7              # first j-tile that can contain the frame-A mask boundary

# consts column layout
C_CBQ = 0            # 6
C_BOUT = 6           # 6
C_B2 = 12            # 6
C_CB1 = 18           # 24
C_SB = 42            # 13  (1 if key allowed for frame B else 0)
C_BB = 55            # 13  (0 / -30 exp bias)
C_SQA = 68           # 13  (1 if key allowed for frame A else 0)
NCONST = 81


def _ln_chunk(nc, pst, prow, psq, env, xblocks, lch, want_rs_row=None, rs_l0=0,
              out_sbcast=False):
    """Column stats for feature-major x blocks (DC x [128, lch] bf16 APs).

    Returns (mu_bf row [1,lch] bf16, rs f32 row AP written into
    want_rs_row[0:1, rs_l0:rs_l0+lch] if given else a fresh row,
    rs_bf row or None if out_sbcast False).
    """
    ones_col, eps1 = env["ones_col"], env["eps1"]
    mu_ps = pst.tile([1, lch], F32, tag="mu", name="mu_ps")
    msq_ps = pst.tile([1, lch], F32, tag="ms", name="msq_ps")
    for dc in range(DC):
        nc.tensor.matmul(mu_ps[:], lhsT=ones_col[:], rhs=xblocks[dc],
                         start=(dc == 0), stop=(dc == DC - 1))
    for dc in range(DC):
        sq = psq.tile([128, lch], BF16, tag="sq", name="sq")
        nc.gpsimd.tensor_mul(sq[:], xblocks[dc], xblocks[dc])
        nc.tensor.matmul(msq_ps[:], lhsT=ones_col[:], rhs=sq[:],
                         start=(dc == 0), stop=(dc == DC - 1))
    mu_bf = prow.tile([1, lch], BF16, tag="mubf", name="mu_bf")
    nc.scalar.mul(mu_bf[:], mu_ps[:], 1.0 / D)
    msq = prow.tile([1, lch], F32, tag="msq", name="msq")
    nc.scalar.mul(msq[:], msq_ps[:], 1.0 / D)
    var = prow.tile([1, lch], F32, tag="var", name="var")
    nc.vector.tensor_mul(var[:], mu_bf[:], mu_bf[:])
    nc.vector.tensor_sub(var[:], msq[:], var[:])
    sd = prow.tile([1, lch], F32, tag="sd", name="sd")
    nc.scalar.activation(sd[:], var[:], AF.Sqrt, bias=eps1[0:1, 0:1])
    if want_rs_row is not None:
        rs_ap = want_rs_row[0:1, rs_l0:rs_l0 + lch]
    else:
        rs_t = prow.tile([1, lch], F32, tag="rsf", name="rs_f")
        rs_ap = rs_t[:]
    nc.vector.reciprocal(rs_ap, sd[:])
    rs_bf = None
    if out_sbcast:
        rs_bf = prow.tile([1, lch], BF16, tag="rsbf", name="rs_bf")
        nc.scalar.mul(rs_bf[:], rs_ap, 1.0)
    return mu_bf, rs_ap, rs_bf


def build_program():
    nc = bacc.Bacc("TRN2")
    xall_d = nc.declare_dram_parameter("xall", [128, DC * L], BF16, isOutput=False)
    xq_d = nc.declare_dram_parameter("xq", [128, DC * NQ], BF16, isOutput=False)
    wq_d = nc.declare_dram_parameter("wq", [128, DC * 3 * D], BF16, isOutput=False)
    wout_d = nc.declare_dram_parameter("wout", [128, DC * D], BF16, isOutput=False)
    w1_d = nc.declare_dram_parameter("w1", [128, DC * DFF], BF16, isOutput=False)
    w2_d = nc.declare_dram_parameter("w2", [128, FT * D], BF16, isOutput=False)
    consts_d = nc.declare_dram_parameter("consts", [128, NCONST], F32, isOutput=False)
    out_d = nc.declare_dram_parameter("out", [NQ, D], F32, isOutput=True)

    env = {}
    with tile.TileContext(nc) as tc, ExitStack() as top:
        pc = top.enter_context(tc.tile_pool(name="const", bufs=1))
        consts = pc.tile([128, NCONST], F32, tag="consts", name="consts")
        nc.sync.dma_start(consts[:], consts_d[:])
        ones_col = pc.tile([128, 1], BF16, tag="onc", name="ones_col")
        nc.vector.memset(ones_col[:], 1.0)
        ones_colf = pc.tile([128, 1], F32, tag="oncf", name="ones_colf")
        nc.vector.memset(ones_colf[:], 1.0)
        ones_row = pc.tile([1, 128], BF16, tag="onr", name="ones_row")
        nc.vector.memset(ones_row[:], 1.0)
        ones_rowf = pc.tile([1, 128], F32, tag="onrf", name="ones_rowf")
        nc.vector.memset(ones_rowf[:], 1.0)
        eps1 = pc.tile([1, 1], F32, tag="eps", name="eps1")
        nc.vector.memset(eps1[:], EPS)
        ident = pc.tile([128, 128], F32, tag="ident", name="ident")
        make_identity(nc, ident[:])
        rs_row = pc.tile([1, NJT * 128], F32, tag="rsrow", name="rs_row")
        rsc = pc.tile([128, NJT], F32, tag="rsc", name="rsc")
        sc = pc.tile([128, NJT], F32, tag="sc", name="sc")
        env.update(ones_col=ones_col, eps1=eps1)

        pnq = top.enter_context(tc.tile_pool(name="nqp", bufs=1))
        nqT = pnq.tile([128, DC * NQ], BF16, tag="nq", name="nqT")
        px2 = top.enter_context(tc.tile_pool(name="x2p", bufs=DC))
        x2T = [px2.tile([128, NQ], F32, tag="x2", name=f"x2T{i}") for i in range(DC)]
        pont = top.enter_context(tc.tile_pool(name="ontp", bufs=DC))
        ONT = [pont.tile([128, NQ], BF16, tag="ont", name=f"ONT{i}") for i in range(DC)]
        pn2 = top.enter_context(tc.tile_pool(name="n2p", bufs=DC))
        n2T = [pn2.tile([128, NQ], BF16, tag="n2", name=f"n2T{i}") for i in range(DC)]
        pout = top.enter_context(tc.tile_pool(name="outp", bufs=DC))
        outT = [pout.tile([128, NQ], F32, tag="ot", name=f"outT{i}") for i in range(DC)]
        prow = top.enter_context(tc.tile_pool(name="rows", bufs=1))
        pwA = top.enter_context(tc.tile_pool(name="wAp", bufs=1))
        wout = pwA.tile([128, DC * D], BF16, tag="wo", name="wout")
        w2 = pwA.tile([128, FT * D], BF16, tag="w2", name="w2")
        pkt = top.enter_context(tc.tile_pool(name="ktp", bufs=DC))
        KT = [pkt.tile([128, L], BF16, tag="kt", name=f"KT{i}") for i in range(DC)]
        pqt = top.enter_context(tc.tile_pool(name="qtp", bufs=DC))
        QT = [pqt.tile([128, NQ], BF16, tag="qt", name=f"QT{i}") for i in range(DC)]
        pva = top.enter_context(tc.tile_pool(name="vap", bufs=NJT))
        VA = [pva.tile([128, H * 65], BF16, tag="va", name=f"VA{i}") for i in range(NJT)]
        ppt = top.enter_context(tc.tile_pool(name="ptp", bufs=2))

        # short-lived inputs on the right allocator stack (freed mid-program)
        s_qkv = ExitStack()   # xall (rewritten in place to x-mu), wq; dies after hp=0
        s_ln = ExitStack()    # xq + square scratch; dies after K-GEMM

        pqkv = s_qkv.enter_context(tc.tile_pool(name="qkvp", bufs=1, side="right"))
        xall = pqkv.tile([128, DC * L], BF16, tag="xa", name="xall")
        nc.sync.dma_start(xall[:], xall_d[:])
        wq = pqkv.tile([128, DC * 3 * D], BF16, tag="wq", name="wq")
        nc.gpsimd.dma_start(wq[:], wq_d[:])
        nc.gpsimd.dma_start(wout[:], wout_d[:])
        nc.gpsimd.dma_start(w2[:], w2_d[:])
        pxq = s_ln.enter_context(tc.tile_pool(name="xqp", bufs=1, side="right"))
        xq = pxq.tile([128, DC * NQ], BF16, tag="xq", name="xq")
        nc.sync.dma_start(xq[:], xq_d[:])

        # nT[dc] = (x - mu), written in place over xall
        nT = [xall[:, dc * L:(dc + 1) * L] for dc in range(DC)]

        # V+ones: preset whole tiles to 1.0; V evac writes the 64-wide blocks
        for lt in range(NJT):
            nc.gpsimd.memset(VA[lt][:], 1.0)

        # ---------------- LN1 (no rs on K path) ----------------
        with ExitStack() as s:
            pst = s.enter_context(tc.tile_pool(name="stps", bufs=2, space="PSUM"))
            pbc = s.enter_context(tc.tile_pool(name="bcps", bufs=2, space="PSUM"))
            psq = s.enter_context(tc.tile_pool(name="sqp", bufs=3))
            for ci in range(4):
                lch = LCH[ci]
                l0 = ci * 512
                xb = [xall[:, dc * L + l0:dc * L + l0 + lch] for dc in range(DC)]
                mu_bf, _, _ = _ln_chunk(nc, pst, prow, psq, env, xb, lch,
                                        want_rs_row=rs_row, rs_l0=l0)
                mub = pbc.tile([128, lch], F32, tag="bc", name="mub")
                nc.tensor.matmul(mub[:], lhsT=ones_row[:], rhs=mu_bf[:],
                                 start=True, stop=True)
                for dc in range(DC):
                    nc.vector.tensor_sub(xb[dc], xb[dc], mub[:])

            # q slice: full normalize (mu and rs)
            xqb = [xq[:, dc * NQ:(dc + 1) * NQ] for dc in range(DC)]
            mu_bfq, _, rs_bfq = _ln_chunk(nc, pst, prow, psq, env, xqb, NQ,
                                          out_sbcast=True)
            mubq = pbc.tile([128, NQ], F32, tag="bc", name="mubq")
            nc.tensor.matmul(mubq[:], lhsT=ones_row[:], rhs=mu_bfq[:],
                             start=True, stop=True)
            sbq = pbc.tile([128, NQ], F32, tag="bc", name="sbq")
            nc.tensor.matmul(sbq[:], lhsT=ones_row[:], rhs=rs_bfq[:],
                             start=True, stop=True)
            for dc in range(DC):
                tmp = psq.tile([128, NQ], F32, tag="tmq", name="tmq")
                nc.vector.tensor_sub(tmp[:], xqb[dc], mubq[:])
                nc.vector.tensor_mul(nqT[:, dc * NQ:(dc + 1) * NQ], tmp[:], sbq[:])

            # rs transposed to partitions: rsc[p, jt] = rs[jt*128+p]; sc = rsc*sb
            # (contraction-1 matmuls: out[128,1] = rs_slice[1,128].T @ ones[1,1])
            nc.vector.memset(rs_row[0:1, L:NJT * 128], 0.0)
            rsT_ps = pbc.tile([128, 16], F32, tag="bc", name="rsT_ps")
            for jt in range(NJT):
                nc.tensor.matmul(rsT_ps[:, jt:jt + 1],
                                 lhsT=rs_row[0:1, jt * 128:(jt + 1) * 128],
                                 rhs=ones_rowf[0:1, 0:1],
                                 start=True, stop=True, skip_group_check=True)
            nc.vector.tensor_copy(rsc[:, 0:NJT], rsT_ps[:, 0:NJT])
            nc.vector.tensor_mul(sc[:, 0:NJT], rsc[:, 0:NJT],
                                 consts[:, C_SB:C_SB + NJT])

            # ---------------- Q and K GEMMs ----------------
            pk = s.enter_context(tc.tile_pool(name="kps", bufs=2, space="PSUM"))
            for et in range(DC):
                ps_q = pk.tile([128, NQ], F32, tag="k", name="ps_q")
                for dc in range(DC):
                    nc.tensor.matmul(
                        ps_q[:],
                        lhsT=wq[:, dc * 3 * D + et * 128:dc * 3 * D + (et + 1) * 128],
                        rhs=nqT[:, dc * NQ:(dc + 1) * NQ],
                        start=(dc == 0), stop=(dc == DC - 1))
                nc.scalar.activation(QT[et][:], ps_q[:], AF.Identity,
                                     bias=consts[:, C_CBQ + et:C_CBQ + et + 1])
            for et in range(DC):
                for ci in range(4):
                    lch = LCH[ci]
                    l0 = ci * 512
                    ps_k = pk.tile([128, lch], F32, tag="k", name="ps_k")
                    for dc in range(DC):
                        nc.tensor.matmul(
                            ps_k[:],
                            lhsT=wq[:, dc * 3 * D + D + et * 128:
                                    dc * 3 * D + D + (et + 1) * 128],
                            rhs=xall[:, dc * L + l0:dc * L + l0 + lch],
                            start=(dc == 0), stop=(dc == DC - 1))
                    nc.vector.tensor_copy(KT[et][:, l0:l0 + lch], ps_k[:])
        s_ln.close()

        # ---------------- attention ----------------
        with ExitStack() as s:
            psc = s.enter_context(tc.tile_pool(name="scps", bufs=2, space="PSUM"))
            pso = s.enter_context(tc.tile_pool(name="sops", bufs=1, space="PSUM"))
            pbb = None

            def attn_hp(hp, with_v):
                o_ps = [pso.tile([128, NQ], F32, tag=f"o{hi}", name=f"o_ps{hi}")
                        for hi in range(2)]
                for jt in range(NJT):
                    jsz = JSZ[jt]
                    if with_v:
                        psv = pps_v.tile([128, D], F32, tag="psv", name="psv")
                        for dc in range(DC):
                            lhsT = xall[:, dc * L + jt * 128:dc * L + jt * 128 + jsz]
                            wv0 = dc * 3 * D + 2 * D
                            nc.tensor.matmul(psv[0:jsz, 0:512], lhsT=lhsT,
                                             rhs=wq[:, wv0:wv0 + 512],
                                             start=(dc == 0), stop=(dc == DC - 1),
                                             skip_group_check=True)
                            nc.tensor.matmul(psv[0:jsz, 512:D], lhsT=lhsT,
                                             rhs=wq[:, wv0 + 512:wv0 + D],
                                             start=(dc == 0), stop=(dc == DC - 1),
                                             skip_group_check=True)
                        vav = VA[jt][:].rearrange("p (h c) -> p h c", c=65)
                        nc.vector.tensor_scalar_mul(
                            vav[0:jsz, :, 0:64],
                            psv[0:jsz, :].rearrange("p (h c) -> p h c", c=64),
                            rsc[0:jsz, jt:jt + 1])
                    smt = psc.tile([128, 1024], F32, tag="smt", name="smt")
                    for hi in range(2):
                        part = 64 * hi
                        nc.tensor.matmul(
                            smt[0:jsz, 512 * hi:512 * hi + NQ],
                            lhsT=KT[hp][part:part + 64, jt * 128:jt * 128 + jsz],
                            rhs=QT[hp][part:part + 64, :],
                            start=True, stop=True, skip_group_check=True)
                    pt = ppt.tile([128, 2 * NQ], BF16, tag="pt", name="pt")
                    smt_v = smt[0:jsz].rearrange("p (b c) -> p b c", c=512)
                    pt_v = pt[0:jsz].rearrange("p (b c) -> p b c", c=NQ)
                    nc.scalar.activation(pt_v[:, :, 0:NQ], smt_v[:, :, 0:NQ],
                                         AF.Exp,
                                         bias=consts[0:jsz, C_BB + jt:C_BB + jt + 1],
                                         scale=sc[0:jsz, jt:jt + 1])
                    if jt >= JLO:
                        nc.vector.tensor_scalar_mul(
                            pt_v[:, :, 0:NPATCH], pt_v[:, :, 0:NPATCH],
                            consts[0:jsz, C_SQA + jt:C_SQA + jt + 1])
                    for hi in range(2):
                        h = 2 * hp + hi
                        nc.tensor.matmul(
                            o_ps[hi][0:65, :],
                            lhsT=VA[jt][0:jsz, h * 65:(h + 1) * 65],
                            rhs=pt[0:jsz, hi * NQ:(hi + 1) * NQ],
                            start=(jt == 0), stop=(jt == NJT - 1),
                            skip_group_check=True)
                return o_ps

            def attn_norm(hp, o_ps):
                for hi in range(2):
                    rr = prow.tile([1, NQ], F32, tag="rr", name="rr")
                    nc.vector.reciprocal(rr[:], o_ps[hi][64:65, :])
                    rb = pbb.tile([64, NQ], F32, tag="rb", name="rb")
                    nc.tensor.matmul(rb[:], lhsT=ones_rowf[0:1, 0:64], rhs=rr[:],
                                     start=True, stop=True)
                    rb_sb = ppt.tile([64, NQ], F32, tag="rbs", name="rb_sb")
                    nc.vector.tensor_copy(rb_sb[:], rb[:])
                    nc.vector.tensor_mul(ONT[hp][64 * hi:64 * hi + 64, :],
                                         o_ps[hi][0:64, :], rb_sb[:])

            with ExitStack() as sv:
                pps_v = sv.enter_context(
                    tc.tile_pool(name="vps", bufs=1, space="PSUM"))
                o_ps0 = attn_hp(0, with_v=True)
            s_qkv.close()   # frees xall (nT) and wq
            pw1 = top.enter_context(tc.tile_pool(name="w1p", bufs=1))
            w1 = pw1.tile([128, DC * DFF], BF16, tag="w1", name="w1")
            nc.gpsimd.dma_start(w1[:], w1_d[:])
            pbb = s.enter_context(tc.tile_pool(name="rbps", bufs=2, space="PSUM"))
            attn_norm(0, o_ps0)
            for hp in range(1, DC):
                o_ps = attn_hp(hp, with_v=False)
                attn_norm(hp, o_ps)

        # ---------------- out-projection + LN2 ----------------
        with ExitStack() as s:
            pop = s.enter_context(tc.tile_pool(name="opps", bufs=2, space="PSUM"))
            for dt in range(DC):
                ps = pop.tile([128, NQ], F32, tag="op", name="ps_o")
                for et in range(DC):
                    nc.tensor.matmul(
                        ps[:], lhsT=wout[:, et * D + dt * 128:et * D + (dt + 1) * 128],
                        rhs=ONT[et][:], start=(et == 0), stop=(et == DC - 1))
                nc.vector.scalar_tensor_tensor(
                    x2T[dt][:], ps[:], consts[:, C_BOUT + dt:C_BOUT + dt + 1],
                    nqT[:, dt * NQ:(dt + 1) * NQ], op0=ALU.add, op1=ALU.add)

            pst2 = s.enter_context(tc.tile_pool(name="st2ps", bufs=1, space="PSUM"))
            pbc2 = s.enter_context(tc.tile_pool(name="bc2ps", bufs=2, space="PSUM"))
            psq2 = s.enter_context(tc.tile_pool(name="sq2p", bufs=2))
            mu_ps = pst2.tile([1, NQ], F32, tag="mu2", name="mu2_ps")
            msq_ps = pst2.tile([1, NQ], F32, tag="ms2", name="msq2_ps")
            for dc in range(DC):
                nc.tensor.matmul(mu_ps[:], lhsT=ones_colf[:], rhs=x2T[dc][:],
                                 start=(dc == 0), stop=(dc == DC - 1))
            for dc in range(DC):
                sq = psq2.tile([128, NQ], BF16, tag="sq2", name="sq2")
                nc.gpsimd.tensor_mul(sq[:], x2T[dc][:], x2T[dc][:])
                nc.tensor.matmul(msq_ps[:], lhsT=ones_col[:], rhs=sq[:],
                                 start=(dc == 0), stop=(dc == DC - 1))
            mu_bf = prow.tile([1, NQ], BF16, tag="mubf", name="mu2_bf")
            nc.scalar.mul(mu_bf[:], mu_ps[:], 1.0 / D)
            msq = prow.tile([1, NQ], F32, tag="msq", name="msq2")
            nc.scalar.mul(msq[:], msq_ps[:], 1.0 / D)
            var = prow.tile([1, NQ], F32, tag="var", name="var2")
            nc.vector.tensor_mul(var[:], mu_bf[:], mu_bf[:])
            nc.vector.tensor_sub(var[:], msq[:], var[:])
            sd = prow.tile([1, NQ], F32, tag="sd", name="sd2")
            nc.scalar.activation(sd[:], var[:], AF.Sqrt, bias=eps1[0:1, 0:1])
            rs2 = prow.tile([1, NQ], F32, tag="rsf", name="rs2")
            nc.vector.reciprocal(rs2[:], sd[:])
            rs2_bf = prow.tile([1, NQ], BF16, tag="rsbf", name="rs2_bf")
            nc.scalar.mul(rs2_bf[:], rs2[:], 1.0)
            mub2 = pbc2.tile([128, NQ], F32, tag="bc2", name="mub2")
            nc.tensor.matmul(mub2[:], lhsT=ones_row[:], rhs=mu_bf[:],
                             start=True, stop=True)
            sb2 = pbc2.tile([128, NQ], F32, tag="bc2", name="sb2")
            nc.tensor.matmul(sb2[:], lhsT=ones_row[:], rhs=rs2_bf[:],
                             start=True, stop=True)
            for dc in range(DC):
                tmp = psq2.tile([128, NQ], F32, tag="tm2", name="tm2")
                nc.vector.tensor_sub(tmp[:], x2T[dc][:], mub2[:])
                nc.vector.tensor_mul(n2T[dc][:], tmp[:], sb2[:])

        # ---------------- FFN ----------------
        with ExitStack() as s:
            pacc = s.enter_context(tc.tile_pool(name="accps", bufs=DC, space="PSUM"))
            pm1 = s.enter_context(tc.tile_pool(name="m1ps", bufs=2, space="PSUM"))
            pff = s.enter_context(tc.tile_pool(name="ffp", bufs=3))
            ps_acc = [pacc.tile([128, NQ], F32, tag="acc", name=f"acc{i}")
                      for i in range(DC)]
            for ft in range(FT):
                ps1 = pm1.tile([128, NQ], F32, tag="m1", name="ps1")
                for dc in range(DC):
                    nc.tensor.matmul(
                        ps1[:],
                        lhsT=w1[:, dc * DFF + ft * 128:dc * DFF + (ft + 1) * 128],
                        rhs=n2T[dc][:], start=(dc == 0), stop=(dc == DC - 1))
                sig = pff.tile([128, NQ], BF16, tag="sig", name="sig")
                nc.scalar.activation(sig[:], ps1[:], AF.Sigmoid,
                                     bias=consts[:, C_CB1 + ft:C_CB1 + ft + 1])
                ffs = pff.tile([128, NQ], BF16, tag="ffs", name="ffs")
                nc.vector.scalar_tensor_tensor(
                    ffs[:], ps1[:], consts[:, C_CB1 + ft:C_CB1 + ft + 1], sig[:],
                    op0=ALU.add, op1=ALU.mult)
                for dt in range(DC):
                    nc.tensor.matmul(
                        ps_acc[dt][:],
                        lhsT=w2[:, ft * D + dt * 128:ft * D + (dt + 1) * 128],
                        rhs=ffs[:], start=(ft == 0), stop=(ft == FT - 1),
                        skip_group_check=True)
            for dt in range(DC):
                nc.vector.scalar_tensor_tensor(
                    outT[dt][:], ps_acc[dt][:], consts[:, C_B2 + dt:C_B2 + dt + 1],
                    x2T[dt][:], op0=ALU.add, op1=ALU.add)

        # ---------------- store (transpose to token-major) ----------------
        with ExitStack() as s:
            ptr2 = s.enter_context(tc.tile_pool(name="trps2", bufs=2, space="PSUM"))
            posb = s.enter_context(tc.tile_pool(name="osbp", bufs=2))
            QSZ = [128, 128, 128, 8]
            for qt in range(4):
                qsz = QSZ[qt]
                osb = posb.tile([128, D], F32, tag="osb", name="osb")
                for dt in range(DC):
                    tp = ptr2.tile([128, 128], F32, tag="tp", name="tp")
                    nc.tensor.transpose(tp[0:qsz, :],
                                        outT[dt][:, qt * 128:qt * 128 + qsz],
                                        ident[:])
                    if dt % 2 == 0:
                        nc.scalar.copy(osb[0:qsz, dt * 128:(dt + 1) * 128],
                                       tp[0:qsz, :])
                    else:
                        nc.vector.tensor_copy(osb[0:qsz, dt * 128:(dt + 1) * 128],
                                              tp[0:qsz, :])
                nc.sync.dma_start(out_d[qt * 128:qt * 128 + qsz, :], osb[0:qsz, :])

    nc.finalize()
    return nc


_NC = None


def _get_nc():
    global _NC
    if _NC is None:
        _NC = build_program()
    return _NC


def _stripes(mat, nstripe):
    """[nstripe*128, C] -> [128, nstripe*C] with stripe i at cols [i*C,(i+1)*C)."""
    r, c = mat.shape
    assert r == nstripe * 128
    return np.ascontiguousarray(
        mat.reshape(nstripe, 128, c).transpose(1, 0, 2).reshape(128, nstripe * c))


def _host_prepare(inputs):
    f32 = np.float32
    bf16 = ml_dtypes.bfloat16
    x = np.asarray(inputs["x"], f32)
    memory = np.asarray(inputs["memory"], f32)
    w_qkv = np.asarray(inputs["w_qkv"], f32)
    w_out = np.asarray(inputs["w_out"], f32)
    b_out = np.asarray(inputs["b_out"], f32)
    g_att = np.asarray(inputs["ln_att_g"], f32)
    b_att = np.asarray(inputs["ln_att_b"], f32)
    g2 = np.asarray(inputs["ln2_g"], f32)
    bb2 = np.asarray(inputs["ln2_b"], f32)
    w1 = np.asarray(inputs["w1"], f32)
    b1 = np.asarray(inputs["b1"], f32)
    w2 = np.asarray(inputs["w2"], f32)
    b2v = np.asarray(inputs["b2"], f32)

    qscale = f32(DH ** -0.5)
    w_qkv_eff = w_qkv * g_att[None, :]
    w_qkv_eff[:D] *= qscale
    cb_qkv = w_qkv @ b_att
    cb_q = (cb_qkv[:D] * qscale).astype(f32)
    cb_v = cb_qkv[2 * D:].astype(f32)
    b_out_eff = (b_out + w_out @ cb_v).astype(f32)
    w1_eff = w1 * g2[None, :]
    cb1_eff = (w1 @ bb2 + b1).astype(f32)

    def cols(v):
        return np.ascontiguousarray(v.reshape(-1, 128).T)

    shared = {
        "wq": _stripes(np.ascontiguousarray(w_qkv_eff.T), DC).astype(bf16),
        "wout": _stripes(np.ascontiguousarray(w_out.T), DC).astype(bf16),
        "w1": _stripes(np.ascontiguousarray(w1_eff.T), DC).astype(bf16),
        "w2": _stripes(np.ascontiguousarray(w2.T), FT).astype(bf16),
    }
    cpart = np.zeros((128, NCONST), f32)
    cpart[:, C_CBQ:C_CBQ + DC] = cols(cb_q)
    cpart[:, C_BOUT:C_BOUT + DC] = cols(b_out_eff)
    cpart[:, C_B2:C_B2 + DC] = cols(b2v)
    cpart[:, C_CB1:C_CB1 + FT] = cols(cb1_eff)

    in_maps = []
    for c in range(NCORES):
        b, hf = divmod(c, 2)
        x_aug = np.concatenate([memory[b, :T], x[b]], axis=0)      # [L, D]
        q0 = T + hf * NQ
        LcA = (5 + 2 * hf) * NPATCH
        LcB = (6 + 2 * hf) * NPATCH
        j = np.arange(NJT * 128)
        sb = ((j < LcB) & (j < L)).astype(f32)
        bb = np.where(sb > 0, 0.0, -30.0).astype(f32)
        sqa = (j < LcA).astype(f32)
        cc = cpart.copy()
        cc[:, C_SB:C_SB + NJT] = sb.reshape(NJT, 128).T
        cc[:, C_BB:C_BB + NJT] = bb.reshape(NJT, 128).T
        cc[:, C_SQA:C_SQA + NJT] = sqa.reshape(NJT, 128).T
        in_maps.append({
            "xall": _stripes(np.ascontiguousarray(x_aug.T), DC).astype(bf16),
            "xq": _stripes(np.ascontiguousarray(x_aug[q0:q0 + NQ].T), DC).astype(bf16),
            "consts": cc,
            **shared,
        })
    return in_maps


def _assemble(results):
    out = np.zeros((B, T, D), np.float32)
    for c in range(NCORES):
        b, hf = divmod(c, 2)
        out[b, hf * NQ:(hf + 1) * NQ, :] = results[c]["out"]
    return out


def kernel(**inputs):
    nc = _get_nc()
    in_maps = _host_prepare(inputs)
    res = run_bass_kernel_spmd(nc, in_maps, list(range(NCORES)))
    return _assemble(res.results)


def _ensure_ntff_hook():
    """Provide antenv.axon_hooks (absent in this image) so trace=True can
    drive NTFF capture through libaxon_pjrt.so, mirroring trn_boot.py."""
    import contextlib
    import ctypes
    import types

    try:
        from antenv.axon_hooks import get_axon_ntff_profile_hook  # noqa: F401
        return
    except ImportError:
        pass
    import antenv

    so_path = "/opt/axon/libaxon_pjrt.so"
    lib = ctypes.CDLL(so_path)
    if not hasattr(lib, "axon_start_nrt_profile"):
        raise RuntimeError("libaxon_pjrt.so lacks NTFF profile symbols")
    lib.axon_start_nrt_profile.argtypes = [ctypes.POINTER(ctypes.c_int64),
                                           ctypes.c_size_t]
    lib.axon_start_nrt_profile.restype = ctypes.c_int64
    lib.axon_stop_nrt_profile.argtypes = [ctypes.c_char_p]
    lib.axon_stop_nrt_profile.restype = ctypes.c_int64

    @contextlib.contextmanager
    def _hook(output_dir, device_ids):
        import jax
        jax.devices()
        if device_ids:
            ids = (ctypes.c_int64 * len(device_ids))(*device_ids)
            rc = lib.axon_start_nrt_profile(ids, len(device_ids))
        else:
            rc = lib.axon_start_nrt_profile(None, 0)
        if rc != 0:
            raise RuntimeError(f"axon_start_nrt_profile rc={rc}")
        try:
            yield
        finally:
            n = lib.axon_stop_nrt_profile(str(output_dir).encode())
            print(f"ntff profile: {n} file(s) written to {output_dir}",
                  file=sys.stderr)

    box = {"h": _hook}
    mod = types.ModuleType("antenv.axon_hooks")
    mod.set_axon_ntff_profile_hook = lambda h: box.__setitem__("h", h)
    mod.get_axon_ntff_profile_hook = lambda: box["h"]
    sys.modules["antenv.axon_hooks"] = mod
    antenv.axon_hooks = mod


def kernel_traced(**inputs):
    """Like kernel() but with NTFF profiling; returns (out, exec_time_ns)."""
    import tempfile

    from concourse import bass_utils as _bu
    _ensure_ntff_hook()
    _bu.upload_artifacts = lambda tmpdir: f"local:{tmpdir}"  # no bucket creds here
    nc = _get_nc()
    in_maps = _host_prepare(inputs)
    tmpdir = tempfile.mkdtemp(prefix="ntff_")
    res = run_bass_kernel_spmd(nc, in_maps, list(range(NCORES)), trace=True,
                               tmpdir=tmpdir)
    return _assemble(res.results), res.exec_time_ns


# revision 28
# speedup vs baseline: 1.0613x; 1.0613x over previous
"""Trainium2 Bass kernel: LookupTransformerBlock (block-causal sparse attention).

Reference semantics (B=4, T=784, D=768, H=12, Dh=64, d_ff=3072):
  x_aug = LN1(concat(memory[:, :T], x))              # [B, 2T, D], ln1 g=1/b=0
  h     = LN_att(x_aug)  (== x_aug up to O(eps) since x_aug is normalized)
  qkv   = h @ w_qkv.T ; block-causal attention over frames of 196
  x2    = x_aug + attn_out
  out   = (x2 + FFN(LN2(x2)))[:, T:, :]

Sharding: 8 cores = (batch b in 0..3) x (query-half hf in 0..1); each core
computes its 392 output rows with K/V over all 1568 positions (data-parallel,
no collectives).  One SPMD program; per-core differences (query slice, mask
columns) are carried in input data only.

v2 design (vs fp32 baseline):
  - all big GEMM operands in bf16 (1 cycle/row on PE vs 4 for fp32 LOW_HIGH)
  - ~15 large DMAs total (weights packed into [128, 6*C] stripe tiles host-side)
  - per-token LN scale/shift broadcast via ones-row matmuls into PSUM
    (no DRAM bounce)
  - K kept un-normalized (x - mu); the per-token inv-std rs[j] is folded into
    the Exp scale operand (per-partition scalar) for scores and applied to V
    during PSUM evacuation (tensor_scalar_mul)
  - scores for both 64-row head-halves land in one 2-bank PSUM tile; ONE
    3D-AP Exp per (head-pair, j-tile); frame-A mask boundary applied by a
    vector tensor_scalar_mul zeroing pass instead of a second Exp
  - LN stats squares on the idle GpSimd engine
"""

import os
import sys
from contextlib import ExitStack

import numpy as np
import ml_dtypes

for _p in ("/opt/trn_rl_repo", os.path.expanduser("~/.axon_site/_ro/trn_rl_repo")):
    if os.path.isdir(_p) and _p not in sys.path:
        sys.path.append(_p)

import concourse.bass as bass
import concourse.bacc as bacc
import concourse.mybir as mybir
import concourse.tile as tile
from concourse.bass_utils import run_bass_kernel_spmd
from concourse.masks import make_identity

F32 = mybir.dt.float32
BF16 = mybir.dt.bfloat16
AF = mybir.ActivationFunctionType
ALU = mybir.AluOpType

B = 4
T = 784
D = 768
L = 2 * T            # 1568
NQ = 392             # query rows per core
H = 12
DH = 64
DFF = 3072
NPATCH = 196
DC = D // 128        # 6
FT = DFF // 128      # 24
NJT = 13             # j-tiles over L (12 x 128 + 32)
JSZ = [128] * 12 + [32]
LCH = [512, 512, 512, 32]
EPS = 1e-5
NCORES = 8
JLO = 7              # first j-tile that can contain the frame-A mask boundary

# consts column layout
C_CBQ = 0            # 6
C_BOUT = 6           # 6
C_B2 = 12            # 6
C_CB1 = 18           # 24
C_SB = 42            # 13  (1 if key allowed for frame B else 0)
C_BB = 55            # 13  (0 / -30 exp bias)
C_SQA = 68           # 13  (1 if key allowed for frame A else 0)
NCONST = 81


def _ln_stats(nc, pst, psq, env, xblocks, lch):
    """Column-sum and column-sum-of-squares matmuls for feature-major x
    blocks (DC x [128, lch] bf16 APs).  Returns (mu_ps, msq_ps) PSUM rows."""
    ones_col = env["ones_col"]
    mu_ps = pst.tile([1, lch], F32, tag="mu", name="mu_ps")
    msq_ps = pst.tile([1, lch], F32, tag="ms", name="msq_ps")
    for dc in range(DC):
        nc.tensor.matmul(mu_ps[:], lhsT=ones_col[:], rhs=xblocks[dc],
                         start=(dc == 0), stop=(dc == DC - 1))
    for dc in range(DC):
        sq = psq.tile([128, lch], BF16, tag="sq", name="sq")
        nc.gpsimd.tensor_mul(sq[:], xblocks[dc], xblocks[dc])
        nc.tensor.matmul(msq_ps[:], lhsT=ones_col[:], rhs=sq[:],
                         start=(dc == 0), stop=(dc == DC - 1))
    return mu_ps, msq_ps


def _ln_rows(nc, prow, env, mu_ps, msq_ps, lch, want_rs_row=None, rs_l0=0,
             out_sbcast=False):
    """mu/rs row math from the stats PSUM rows."""
    eps1 = env["eps1"]
    mu_bf = prow.tile([1, lch], BF16, tag="mubf", name="mu_bf")
    nc.scalar.mul(mu_bf[:], mu_ps[:], 1.0 / D)
    msq = prow.tile([1, lch], F32, tag="msq", name="msq")
    nc.scalar.mul(msq[:], msq_ps[:], 1.0 / D)
    var = prow.tile([1, lch], F32, tag="var", name="var")
    nc.vector.tensor_mul(var[:], mu_bf[:], mu_bf[:])
    nc.vector.tensor_sub(var[:], msq[:], var[:])
    sd = prow.tile([1, lch], F32, tag="sd", name="sd")
    nc.scalar.activation(sd[:], var[:], AF.Sqrt, bias=eps1[0:1, 0:1])
    if want_rs_row is not None:
        rs_ap = want_rs_row[0:1, rs_l0:rs_l0 + lch]
    else:
        rs_t = prow.tile([1, lch], F32, tag="rsf", name="rs_f")
        rs_ap = rs_t[:]
    nc.vector.reciprocal(rs_ap, sd[:])
    rs_bf = None
    if out_sbcast:
        rs_bf = prow.tile([1, lch], BF16, tag="rsbf", name="rs_bf")
        nc.scalar.mul(rs_bf[:], rs_ap, 1.0)
    return mu_bf, rs_ap, rs_bf


def build_program():
    nc = bacc.Bacc("TRN2")
    xall_d = nc.declare_dram_parameter("xall", [128, DC * L], BF16, isOutput=False)
    xq_d = nc.declare_dram_parameter("xq", [128, DC * NQ], BF16, isOutput=False)
    wq_d = nc.declare_dram_parameter("wq", [128, DC * 3 * D], BF16, isOutput=False)
    wout_d = nc.declare_dram_parameter("wout", [128, DC * D], BF16, isOutput=False)
    w1_d = nc.declare_dram_parameter("w1", [128, DC * DFF], BF16, isOutput=False)
    w2_d = nc.declare_dram_parameter("w2", [128, FT * D], BF16, isOutput=False)
    consts_d = nc.declare_dram_parameter("consts", [128, NCONST], F32, isOutput=False)
    out_d = nc.declare_dram_parameter("out", [NQ, D], F32, isOutput=True)

    env = {}
    with tile.TileContext(nc) as tc, ExitStack() as top:
        pc = top.enter_context(tc.tile_pool(name="const", bufs=1))
        consts = pc.tile([128, NCONST], F32, tag="consts", name="consts")
        nc.sync.dma_start(consts[:], consts_d[:])
        ones_col = pc.tile([128, 1], BF16, tag="onc", name="ones_col")
        nc.vector.memset(ones_col[:], 1.0)
        ones_colf = pc.tile([128, 1], F32, tag="oncf", name="ones_colf")
        nc.vector.memset(ones_colf[:], 1.0)
        ones_row = pc.tile([1, 128], BF16, tag="onr", name="ones_row")
        nc.vector.memset(ones_row[:], 1.0)
        ones_rowf = pc.tile([1, 128], F32, tag="onrf", name="ones_rowf")
        nc.vector.memset(ones_rowf[:], 1.0)
        eps1 = pc.tile([1, 1], F32, tag="eps", name="eps1")
        nc.vector.memset(eps1[:], EPS)
        ident = pc.tile([128, 128], F32, tag="ident", name="ident")
        make_identity(nc, ident[:])
        rs_row = pc.tile([1, NJT * 128], F32, tag="rsrow", name="rs_row")
        rsc = pc.tile([128, NJT], F32, tag="rsc", name="rsc")
        sc = pc.tile([128, NJT], F32, tag="sc", name="sc")
        env.update(ones_col=ones_col, eps1=eps1)

        pnq = top.enter_context(tc.tile_pool(name="nqp", bufs=1))
        nqT = pnq.tile([128, DC * NQ], BF16, tag="nq", name="nqT")
        px2 = top.enter_context(tc.tile_pool(name="x2p", bufs=DC))
        x2T = [px2.tile([128, NQ], F32, tag="x2", name=f"x2T{i}") for i in range(DC)]
        pont = top.enter_context(tc.tile_pool(name="ontp", bufs=DC))
        ONT = [pont.tile([128, NQ], BF16, tag="ont", name=f"ONT{i}") for i in range(DC)]
        pn2 = top.enter_context(tc.tile_pool(name="n2p", bufs=DC))
        n2T = [pn2.tile([128, NQ], BF16, tag="n2", name=f"n2T{i}") for i in range(DC)]
        pout = top.enter_context(tc.tile_pool(name="outp", bufs=DC))
        outT = [pout.tile([128, NQ], F32, tag="ot", name=f"outT{i}") for i in range(DC)]
        prow = top.enter_context(tc.tile_pool(name="rows", bufs=1))
        pwA = top.enter_context(tc.tile_pool(name="wAp", bufs=1))
        wout = pwA.tile([128, DC * D], BF16, tag="wo", name="wout")
        w2 = pwA.tile([128, FT * D], BF16, tag="w2", name="w2")
        pkt = top.enter_context(tc.tile_pool(name="ktp", bufs=DC))
        KT = [pkt.tile([128, L], BF16, tag="kt", name=f"KT{i}") for i in range(DC)]
        pqt = top.enter_context(tc.tile_pool(name="qtp", bufs=DC))
        QT = [pqt.tile([128, NQ], BF16, tag="qt", name=f"QT{i}") for i in range(DC)]
        pva = top.enter_context(tc.tile_pool(name="vap", bufs=NJT))
        VA = [pva.tile([128, H * 65], BF16, tag="va", name=f"VA{i}") for i in range(NJT)]
        ppt = top.enter_context(tc.tile_pool(name="ptp", bufs=3))

        # short-lived inputs on the right allocator stack (freed mid-program)
        s_qkv = ExitStack()   # xall (rewritten in place to x-mu), wq; dies after hp=0
        s_ln = ExitStack()    # xq + square scratch; dies after K-GEMM

        pqkv = s_qkv.enter_context(tc.tile_pool(name="qkvp", bufs=1, side="right"))
        xall = pqkv.tile([128, DC * L], BF16, tag="xa", name="xall")
        nc.sync.dma_start(xall[:], xall_d[:])
        wq = pqkv.tile([128, DC * 3 * D], BF16, tag="wq", name="wq")
        nc.gpsimd.dma_start(wq[:], wq_d[:])
        pxq = s_ln.enter_context(tc.tile_pool(name="xqp", bufs=1, side="right"))
        xq = pxq.tile([128, DC * NQ], BF16, tag="xq", name="xq")
        nc.sync.dma_start(xq[:], xq_d[:])

        # nT[dc] = (x - mu), written in place over xall
        nT = [xall[:, dc * L:(dc + 1) * L] for dc in range(DC)]

        # ---------------- LN1 (no rs on K path) + Q/K GEMMs ----------------
        # stats run two chunks ahead of the row math so the PE never waits on
        # the scalar/vector row pipeline; K-GEMM for chunk ci follows its
        # normalize immediately.
        with ExitStack() as s:
            pst = s.enter_context(tc.tile_pool(name="stps", bufs=2, space="PSUM"))
            pbc = s.enter_context(tc.tile_pool(name="bcps", bufs=2, space="PSUM"))
            pk = s.enter_context(tc.tile_pool(name="kps", bufs=2, space="PSUM"))
            psq = s.enter_context(tc.tile_pool(name="sqp", bufs=3, side="right"))

            xqb = [xq[:, dc * NQ:(dc + 1) * NQ] for dc in range(DC)]
            xbs = [[xall[:, dc * L + ci * 512:dc * L + ci * 512 + LCH[ci]]
                    for dc in range(DC)] for ci in range(4)]

            def emit_k_gemm(ci):
                lch = LCH[ci]
                l0 = ci * 512
                for et in range(DC):
                    ps_k = pk.tile([128, lch], F32, tag="k", name="ps_k")
                    for dc in range(DC):
                        nc.tensor.matmul(
                            ps_k[:],
                            lhsT=wq[:, dc * 3 * D + D + et * 128:
                                    dc * 3 * D + D + (et + 1) * 128],
                            rhs=xall[:, dc * L + l0:dc * L + l0 + lch],
                            start=(dc == 0), stop=(dc == DC - 1))
                    nc.vector.tensor_copy(KT[et][:, l0:l0 + lch], ps_k[:])

            stq = [None] * 5
            stq[0] = _ln_stats(nc, pst, psq, env, xbs[0], LCH[0])
            stq[1] = _ln_stats(nc, pst, psq, env, xbs[1], LCH[1])
            for ci in range(4):
                lch = LCH[ci]
                l0 = ci * 512
                mu_bf, _, _ = _ln_rows(nc, prow, env, *stq[ci], lch,
                                       want_rs_row=rs_row, rs_l0=l0)
                mub = pbc.tile([128, lch], F32, tag="bc", name="mub")
                nc.tensor.matmul(mub[:], lhsT=ones_row[:], rhs=mu_bf[:],
                                 start=True, stop=True)
                for dc in range(DC):
                    nc.vector.tensor_sub(xbs[ci][dc], xbs[ci][dc], mub[:])
                if ci + 2 < 4:
                    stq[ci + 2] = _ln_stats(nc, pst, psq, env, xbs[ci + 2],
                                            LCH[ci + 2])
                elif ci == 2:
                    stq[4] = _ln_stats(nc, pst, psq, env, xqb, NQ)
                emit_k_gemm(ci)

            # q slice: full normalize (mu and rs)
            mu_bfq, _, rs_bfq = _ln_rows(nc, prow, env, *stq[4], NQ,
                                         out_sbcast=True)
            mubq = pbc.tile([128, NQ], F32, tag="bc", name="mubq")
            nc.tensor.matmul(mubq[:], lhsT=ones_row[:], rhs=mu_bfq[:],
                             start=True, stop=True)
            sbq = pbc.tile([128, NQ], F32, tag="bc", name="sbq")
            nc.tensor.matmul(sbq[:], lhsT=ones_row[:], rhs=rs_bfq[:],
                             start=True, stop=True)
            for dc in range(DC):
                tmp = psq.tile([128, NQ], F32, tag="tmq", name="tmq")
                nc.vector.tensor_sub(tmp[:], xqb[dc], mubq[:])
                nc.vector.tensor_mul(nqT[:, dc * NQ:(dc + 1) * NQ], tmp[:], sbq[:])

            # rs transposed to partitions: rsc[p, jt] = rs[jt*128+p]; sc = rsc*sb
            # (contraction-1 matmuls: out[128,1] = rs_slice[1,128].T @ ones[1,1])
            nc.vector.memset(rs_row[0:1, L:NJT * 128], 0.0)
            rsT_ps = pbc.tile([128, 16], F32, tag="bc", name="rsT_ps")
            for jt in range(NJT):
                nc.tensor.matmul(rsT_ps[:, jt:jt + 1],
                                 lhsT=rs_row[0:1, jt * 128:(jt + 1) * 128],
                                 rhs=ones_rowf[0:1, 0:1],
                                 start=True, stop=True, skip_group_check=True)
            nc.vector.tensor_copy(rsc[:, 0:NJT], rsT_ps[:, 0:NJT])
            nc.vector.tensor_mul(sc[:, 0:NJT], rsc[:, 0:NJT],
                                 consts[:, C_SB:C_SB + NJT])

            # ---------------- Q GEMM ----------------
            for et in range(DC):
                ps_q = pk.tile([128, NQ], F32, tag="k", name="ps_q")
                for dc in range(DC):
                    nc.tensor.matmul(
                        ps_q[:],
                        lhsT=wq[:, dc * 3 * D + et * 128:dc * 3 * D + (et + 1) * 128],
                        rhs=nqT[:, dc * NQ:(dc + 1) * NQ],
                        start=(dc == 0), stop=(dc == DC - 1))
                nc.scalar.activation(QT[et][:], ps_q[:], AF.Identity,
                                     bias=consts[:, C_CBQ + et:C_CBQ + et + 1])
        s_ln.close()

        # V+ones: preset whole tiles to 1.0; V evac writes the 64-wide blocks.
        # Emitted after the LN squares so the gpsimd queue serves those first;
        # likewise the tail-phase weight DMAs load after wq/xall are done.
        for lt in range(NJT):
            nc.gpsimd.memset(VA[lt][:], 1.0)
        nc.gpsimd.dma_start(wout[:], wout_d[:])
        nc.gpsimd.dma_start(w2[:], w2_d[:])

        # ---------------- attention ----------------
        with ExitStack() as s:
            psc = s.enter_context(tc.tile_pool(name="scps", bufs=2, space="PSUM"))
            pso = s.enter_context(tc.tile_pool(name="sops", bufs=1, space="PSUM"))
            pbb = None

            def attn_hp(hp, with_v):
                o_ps = [pso.tile([128, NQ], F32, tag=f"o{hi}", name=f"o_ps{hi}")
                        for hi in range(2)]

                def emit_pv(jt, pt):
                    jsz = JSZ[jt]
                    for hi in range(2):
                        h = 2 * hp + hi
                        nc.tensor.matmul(
                            o_ps[hi][0:65, :],
                            lhsT=VA[jt][0:jsz, h * 65:(h + 1) * 65],
                            rhs=pt[0:jsz, hi * NQ:(hi + 1) * NQ],
                            start=(jt == 0), stop=(jt == NJT - 1),
                            skip_group_check=True)

                lag = None  # (jt, pt): PV lags one j-tile so the PE is never
                for jt in range(NJT):  # blocked on the Exp of the current tile
                    jsz = JSZ[jt]
                    if with_v:
                        psv = pps_v.tile([128, D], F32, tag="psv", name="psv")
                        for dc in range(DC):
                            lhsT = xall[:, dc * L + jt * 128:dc * L + jt * 128 + jsz]
                            wv0 = dc * 3 * D + 2 * D
                            nc.tensor.matmul(psv[0:jsz, 0:512], lhsT=lhsT,
                                             rhs=wq[:, wv0:wv0 + 512],
                                             start=(dc == 0), stop=(dc == DC - 1),
                                             skip_group_check=True)
                            nc.tensor.matmul(psv[0:jsz, 512:D], lhsT=lhsT,
                                             rhs=wq[:, wv0 + 512:wv0 + D],
                                             start=(dc == 0), stop=(dc == DC - 1),
                                             skip_group_check=True)
                        vav = VA[jt][:].rearrange("p (h c) -> p h c", c=65)
                        nc.vector.tensor_scalar_mul(
                            vav[0:jsz, :, 0:64],
                            psv[0:jsz, :].rearrange("p (h c) -> p h c", c=64),
                            rsc[0:jsz, jt:jt + 1])
                    smt = psc.tile([128, 1024], F32, tag="smt", name="smt")
                    for hi in range(2):
                        part = 64 * hi
                        nc.tensor.matmul(
                            smt[0:jsz, 512 * hi:512 * hi + NQ],
                            lhsT=KT[hp][part:part + 64, jt * 128:jt * 128 + jsz],
                            rhs=QT[hp][part:part + 64, :],
                            start=True, stop=True, skip_group_check=True)
                    if lag is not None:
                        emit_pv(*lag)
                    pt = ppt.tile([128, 2 * NQ], BF16, tag="pt", name="pt")
                    smt_v = smt[0:jsz].rearrange("p (b c) -> p b c", c=512)
                    pt_v = pt[0:jsz].rearrange("p (b c) -> p b c", c=NQ)
                    nc.scalar.activation(pt_v[:, :, 0:NQ], smt_v[:, :, 0:NQ],
                                         AF.Exp,
                                         bias=consts[0:jsz, C_BB + jt:C_BB + jt + 1],
                                         scale=sc[0:jsz, jt:jt + 1])
                    if jt >= JLO:
                        nc.vector.tensor_scalar_mul(
                            pt_v[:, :, 0:NPATCH], pt_v[:, :, 0:NPATCH],
                            consts[0:jsz, C_SQA + jt:C_SQA + jt + 1])
                    lag = (jt, pt)
                emit_pv(*lag)
                return o_ps

            def attn_norm(hp, o_ps):
                for hi in range(2):
                    rr = prow.tile([1, NQ], F32, tag="rr", name="rr")
                    nc.vector.reciprocal(rr[:], o_ps[hi][64:65, :])
                    rb = pbb.tile([64, NQ], F32, tag="rb", name="rb")
                    nc.tensor.matmul(rb[:], lhsT=ones_rowf[0:1, 0:64], rhs=rr[:],
                                     start=True, stop=True)
                    rb_sb = ppt.tile([64, NQ], F32, tag="rbs", name="rb_sb")
                    nc.vector.tensor_copy(rb_sb[:], rb[:])
                    nc.vector.tensor_mul(ONT[hp][64 * hi:64 * hi + 64, :],
                                         o_ps[hi][0:64, :], rb_sb[:])

            with ExitStack() as sv:
                pps_v = sv.enter_context(
                    tc.tile_pool(name="vps", bufs=1, space="PSUM"))
                o_ps0 = attn_hp(0, with_v=True)
            s_qkv.close()   # frees xall (nT) and wq
            pw1 = top.enter_context(tc.tile_pool(name="w1p", bufs=1))
            w1 = pw1.tile([128, DC * DFF], BF16, tag="w1", name="w1")
            nc.gpsimd.dma_start(w1[:], w1_d[:])
            pbb = s.enter_context(tc.tile_pool(name="rbps", bufs=2, space="PSUM"))
            attn_norm(0, o_ps0)
            for hp in range(1, DC):
                o_ps = attn_hp(hp, with_v=False)
                attn_norm(hp, o_ps)

        # ---------------- out-projection + LN2 ----------------
        with ExitStack() as s:
            pop = s.enter_context(tc.tile_pool(name="opps", bufs=2, space="PSUM"))
            for dt in range(DC):
                ps = pop.tile([128, NQ], F32, tag="op", name="ps_o")
                for et in range(DC):
                    nc.tensor.matmul(
                        ps[:], lhsT=wout[:, et * D + dt * 128:et * D + (dt + 1) * 128],
                        rhs=ONT[et][:], start=(et == 0), stop=(et == DC - 1))
                nc.vector.scalar_tensor_tensor(
                    x2T[dt][:], ps[:], consts[:, C_BOUT + dt:C_BOUT + dt + 1],
                    nqT[:, dt * NQ:(dt + 1) * NQ], op0=ALU.add, op1=ALU.add)

            pst2 = s.enter_context(tc.tile_pool(name="st2ps", bufs=1, space="PSUM"))
            pbc2 = s.enter_context(tc.tile_pool(name="bc2ps", bufs=2, space="PSUM"))
            psq2 = s.enter_context(tc.tile_pool(name="sq2p", bufs=2))
            mu_ps = pst2.tile([1, NQ], F32, tag="mu2", name="mu2_ps")
            msq_ps = pst2.tile([1, NQ], F32, tag="ms2", name="msq2_ps")
            for dc in range(DC):
                nc.tensor.matmul(mu_ps[:], lhsT=ones_colf[:], rhs=x2T[dc][:],
                                 start=(dc == 0), stop=(dc == DC - 1))
            for dc in range(DC):
                sq = psq2.tile([128, NQ], BF16, tag="sq2", name="sq2")
                nc.gpsimd.tensor_mul(sq[:], x2T[dc][:], x2T[dc][:])
                nc.tensor.matmul(msq_ps[:], lhsT=ones_col[:], rhs=sq[:],
                                 start=(dc == 0), stop=(dc == DC - 1))
            mu_bf = prow.tile([1, NQ], BF16, tag="mubf", name="mu2_bf")
            nc.scalar.mul(mu_bf[:], mu_ps[:], 1.0 / D)
            msq = prow.tile([1, NQ], F32, tag="msq", name="msq2")
            nc.scalar.mul(msq[:], msq_ps[:], 1.0 / D)
            var = prow.tile([1, NQ], F32, tag="var", name="var2")
            nc.vector.tensor_mul(var[:], mu_bf[:], mu_bf[:])
            nc.vector.tensor_sub(var[:], msq[:], var[:])
            sd = prow.tile([1, NQ], F32, tag="sd", name="sd2")
            nc.scalar.activation(sd[:], var[:], AF.Sqrt, bias=eps1[0:1, 0:1])
            rs2 = prow.tile([1, NQ], F32, tag="rsf", name="rs2")
            nc.vector.reciprocal(rs2[:], sd[:])
            rs2_bf = prow.tile([1, NQ], BF16, tag="rsbf", name="rs2_bf")
            nc.scalar.mul(rs2_bf[:], rs2[:], 1.0)
            mub2 = pbc2.tile([128, NQ], F32, tag="bc2", name="mub2")
            nc.tensor.matmul(mub2[:], lhsT=ones_row[:], rhs=mu_bf[:],
                             start=True, stop=True)
            sb2 = pbc2.tile([128, NQ], F32, tag="bc2", name="sb2")
            nc.tensor.matmul(sb2[:], lhsT=ones_row[:], rhs=rs2_bf[:],
                             start=True, stop=True)
            for dc in range(DC):
                tmp = psq2.tile([128, NQ], F32, tag="tm2", name="tm2")
                nc.vector.tensor_sub(tmp[:], x2T[dc][:], mub2[:])
                nc.vector.tensor_mul(n2T[dc][:], tmp[:], sb2[:])

        # ---------------- FFN ----------------
        with ExitStack() as s:
            pacc = s.enter_context(tc.tile_pool(name="accps", bufs=DC, space="PSUM"))
            pm1 = s.enter_context(tc.tile_pool(name="m1ps", bufs=2, space="PSUM"))
            pff = s.enter_context(tc.tile_pool(name="ffp", bufs=3))
            ps_acc = [pacc.tile([128, NQ], F32, tag="acc", name=f"acc{i}")
                      for i in range(DC)]
            for ft in range(FT):
                ps1 = pm1.tile([128, NQ], F32, tag="m1", name="ps1")
                for dc in range(DC):
                    nc.tensor.matmul(
                        ps1[:],
                        lhsT=w1[:, dc * DFF + ft * 128:dc * DFF + (ft + 1) * 128],
                        rhs=n2T[dc][:], start=(dc == 0), stop=(dc == DC - 1))
                sig = pff.tile([128, NQ], BF16, tag="sig", name="sig")
                nc.scalar.activation(sig[:], ps1[:], AF.Sigmoid,
                                     bias=consts[:, C_CB1 + ft:C_CB1 + ft + 1])
                ffs = pff.tile([128, NQ], BF16, tag="ffs", name="ffs")
                nc.vector.scalar_tensor_tensor(
                    ffs[:], ps1[:], consts[:, C_CB1 + ft:C_CB1 + ft + 1], sig[:],
                    op0=ALU.add, op1=ALU.mult)
                for dt in range(DC):
                    nc.tensor.matmul(
                        ps_acc[dt][:],
                        lhsT=w2[:, ft * D + dt * 128:ft * D + (dt + 1) * 128],
                        rhs=ffs[:], start=(ft == 0), stop=(ft == FT - 1),
                        skip_group_check=True)
            for dt in range(DC):
                nc.vector.scalar_tensor_tensor(
                    outT[dt][:], ps_acc[dt][:], consts[:, C_B2 + dt:C_B2 + dt + 1],
                    x2T[dt][:], op0=ALU.add, op1=ALU.add)

        # ---------------- store (transpose to token-major) ----------------
        with ExitStack() as s:
            ptr2 = s.enter_context(tc.tile_pool(name="trps2", bufs=2, space="PSUM"))
            posb = s.enter_context(tc.tile_pool(name="osbp", bufs=2))
            QSZ = [128, 128, 128, 8]
            for qt in range(4):
                qsz = QSZ[qt]
                osb = posb.tile([128, D], F32, tag="osb", name="osb")
                for dt in range(DC):
                    tp = ptr2.tile([128, 128], F32, tag="tp", name="tp")
                    nc.tensor.transpose(tp[0:qsz, :],
                                        outT[dt][:, qt * 128:qt * 128 + qsz],
                                        ident[:])
                    if dt % 2 == 0:
                        nc.scalar.copy(osb[0:qsz, dt * 128:(dt + 1) * 128],
                                       tp[0:qsz, :])
                    else:
                        nc.vector.tensor_copy(osb[0:qsz, dt * 128:(dt + 1) * 128],
                                              tp[0:qsz, :])
                nc.sync.dma_start(out_d[qt * 128:qt * 128 + qsz, :], osb[0:qsz, :])

    nc.finalize()
    return nc


_NC = None


def _get_nc():
    global _NC
    if _NC is None:
        _NC = build_program()
    return _NC


def _stripes(mat, nstripe):
    """[nstripe*128, C] -> [128, nstripe*C] with stripe i at cols [i*C,(i+1)*C)."""
    r, c = mat.shape
    assert r == nstripe * 128
    return np.ascontiguousarray(
        mat.reshape(nstripe, 128, c).transpose(1, 0, 2).reshape(128, nstripe * c))


def _host_prepare(inputs):
    f32 = np.float32
    bf16 = ml_dtypes.bfloat16
    x = np.asarray(inputs["x"], f32)
    memory = np.asarray(inputs["memory"], f32)
    w_qkv = np.asarray(inputs["w_qkv"], f32)
    w_out = np.asarray(inputs["w_out"], f32)
    b_out = np.asarray(inputs["b_out"], f32)
    g_att = np.asarray(inputs["ln_att_g"], f32)
    b_att = np.asarray(inputs["ln_att_b"], f32)
    g2 = np.asarray(inputs["ln2_g"], f32)
    bb2 = np.asarray(inputs["ln2_b"], f32)
    w1 = np.asarray(inputs["w1"], f32)
    b1 = np.asarray(inputs["b1"], f32)
    w2 = np.asarray(inputs["w2"], f32)
    b2v = np.asarray(inputs["b2"], f32)

    qscale = f32(DH ** -0.5)
    w_qkv_eff = w_qkv * g_att[None, :]
    w_qkv_eff[:D] *= qscale
    cb_qkv = w_qkv @ b_att
    cb_q = (cb_qkv[:D] * qscale).astype(f32)
    cb_v = cb_qkv[2 * D:].astype(f32)
    b_out_eff = (b_out + w_out @ cb_v).astype(f32)
    w1_eff = w1 * g2[None, :]
    cb1_eff = (w1 @ bb2 + b1).astype(f32)

    def cols(v):
        return np.ascontiguousarray(v.reshape(-1, 128).T)

    shared = {
        "wq": _stripes(np.ascontiguousarray(w_qkv_eff.T), DC).astype(bf16),
        "wout": _stripes(np.ascontiguousarray(w_out.T), DC).astype(bf16),
        "w1": _stripes(np.ascontiguousarray(w1_eff.T), DC).astype(bf16),
        "w2": _stripes(np.ascontiguousarray(w2.T), FT).astype(bf16),
    }
    cpart = np.zeros((128, NCONST), f32)
    cpart[:, C_CBQ:C_CBQ + DC] = cols(cb_q)
    cpart[:, C_BOUT:C_BOUT + DC] = cols(b_out_eff)
    cpart[:, C_B2:C_B2 + DC] = cols(b2v)
    cpart[:, C_CB1:C_CB1 + FT] = cols(cb1_eff)

    in_maps = []
    for c in range(NCORES):
        b, hf = divmod(c, 2)
        x_aug = np.concatenate([memory[b, :T], x[b]], axis=0)      # [L, D]
        q0 = T + hf * NQ
        LcA = (5 + 2 * hf) * NPATCH
        LcB = (6 + 2 * hf) * NPATCH
        j = np.arange(NJT * 128)
        sb = ((j < LcB) & (j < L)).astype(f32)
        bb = np.where(sb > 0, 0.0, -30.0).astype(f32)
        sqa = (j < LcA).astype(f32)
        cc = cpart.copy()
        cc[:, C_SB:C_SB + NJT] = sb.reshape(NJT, 128).T
        cc[:, C_BB:C_BB + NJT] = bb.reshape(NJT, 128).T
        cc[:, C_SQA:C_SQA + NJT] = sqa.reshape(NJT, 128).T
        in_maps.append({
            "xall": _stripes(np.ascontiguousarray(x_aug.T), DC).astype(bf16),
            "xq": _stripes(np.ascontiguousarray(x_aug[q0:q0 + NQ].T), DC).astype(bf16),
            "consts": cc,
            **shared,
        })
    return in_maps


def _assemble(results):
    out = np.zeros((B, T, D), np.float32)
    for c in range(NCORES):
        b, hf = divmod(c, 2)
        out[b, hf * NQ:(hf + 1) * NQ, :] = results[c]["out"]
    return out


def kernel(**inputs):
    nc = _get_nc()
    in_maps = _host_prepare(inputs)
    res = run_bass_kernel_spmd(nc, in_maps, list(range(NCORES)))
    return _assemble(res.results)


def _ensure_ntff_hook():
    """Provide antenv.axon_hooks (absent in this image) so trace=True can
    drive NTFF capture through libaxon_pjrt.so, mirroring trn_boot.py."""
    import contextlib
    import ctypes
    import types

    try:
        from antenv.axon_hooks import get_axon_ntff_profile_hook  # noqa: F401
        return
    except ImportError:
        pass
    import antenv

    so_path = "/opt/axon/libaxon_pjrt.so"
    lib = ctypes.CDLL(so_path)
    if not hasattr(lib, "axon_start_nrt_profile"):
        raise RuntimeError("libaxon_pjrt.so lacks NTFF profile symbols")
    lib.axon_start_nrt_profile.argtypes = [ctypes.POINTER(ctypes.c_int64),
                                           ctypes.c_size_t]
    lib.axon_start_nrt_profile.restype = ctypes.c_int64
    lib.axon_stop_nrt_profile.argtypes = [ctypes.c_char_p]
    lib.axon_stop_nrt_profile.restype = ctypes.c_int64

    @contextlib.contextmanager
    def _hook(output_dir, device_ids):
        import jax
        jax.devices()
        if device_ids:
            ids = (ctypes.c_int64 * len(device_ids))(*device_ids)
            rc = lib.axon_start_nrt_profile(ids, len(device_ids))
        else:
            rc = lib.axon_start_nrt_profile(None, 0)
        if rc != 0:
            raise RuntimeError(f"axon_start_nrt_profile rc={rc}")
        try:
            yield
        finally:
            n = lib.axon_stop_nrt_profile(str(output_dir).encode())
            print(f"ntff profile: {n} file(s) written to {output_dir}",
                  file=sys.stderr)

    box = {"h": _hook}
    mod = types.ModuleType("antenv.axon_hooks")
    mod.set_axon_ntff_profile_hook = lambda h: box.__setitem__("h", h)
    mod.get_axon_ntff_profile_hook = lambda: box["h"]
    sys.modules["antenv.axon_hooks"] = mod
    antenv.axon_hooks = mod


def kernel_traced(**inputs):
    """Like kernel() but with NTFF profiling; returns (out, exec_time_ns)."""
    import tempfile

    from concourse import bass_utils as _bu
    _ensure_ntff_hook()
    _bu.upload_artifacts = lambda tmpdir: f"local:{tmpdir}"  # no bucket creds here
    nc = _get_nc()
    in_maps = _host_prepare(inputs)
    tmpdir = tempfile.mkdtemp(prefix="ntff_")
    res = run_bass_kernel_spmd(nc, in_maps, list(range(NCORES)), trace=True,
                               tmpdir=tmpdir)
    return _assemble(res.results), res.exec_time_ns
